# revision 1
# baseline (speedup 1.0000x reference)
"""Deformable PS-ROI pooling on Trainium2 (Bass/Tile), SPMD over 8 cores.

Strategy: data-parallel over ROIs (64 rois/core), feature map replicated in
DRAM in channel-last layout so each bilinear corner is one contiguous 1 KiB
gather.  Per (roi, bin) the 4 samples x 4 corners = 16 gathered pixel vectors
land on 16 SBUF partitions (112 per 7-bin chunk); a block-diagonal 0/1 mask
matmul on the PE reduces them into the [49, 256] output, with all bilinear /
validity / 1-over-count factors pre-folded into a per-partition scalar weight.
"""

import numpy as np

import concourse.bass as bass
import concourse.bacc as bacc
import concourse.mybir as mybir
from concourse import tile
from concourse.bass import IndirectOffsetOnAxis
from concourse.bass_utils import run_bass_kernel_spmd

F32 = mybir.dt.float32
F32R = mybir.dt.float32r
I32 = mybir.dt.int32
OP = mybir.AluOpType

N_CORES = 8
R = 64                  # rois per core
P = 7                   # pooled output size
NB = P * P              # 49 bins
CH = 256                # channels
H = W = 128             # feature map spatial
B = 2                   # batch
NPX = B * H * W         # 32768 flat pixels
TPB = 16                # terms (sample x corner) per bin
T = NB * TPB            # 784 terms per roi
KC = 112                # terms per K-chunk = 7 bins
NCH = 7                 # chunks per roi
G = 1                   # rois per gather group (Q7 idx scratch caps ~1024 descs)
SCALE = 0.0625
TRANS_STD = 0.1


def _floor(nc, pool, x, name):
    """floor(x) robust to convert rounding mode: returns (floor_f32, frac)."""
    xi = pool.tile([R, x.shape[1]], I32, tag=name + "_i")
    nc.vector.tensor_copy(xi[:, :], x)
    xf = pool.tile([R, x.shape[1]], F32, tag=name + "_f")
    nc.vector.tensor_copy(xf[:, :], xi[:, :])
    d = pool.tile([R, x.shape[1]], F32, tag=name + "_d")
    nc.vector.tensor_tensor(d[:, :], x, xf[:, :], OP.subtract)
    neg = pool.tile([R, x.shape[1]], F32, tag=name + "_n")
    nc.vector.tensor_scalar(neg[:, :], d[:, :], 0.0, None, OP.is_lt)
    fl = pool.tile([R, x.shape[1]], F32, tag=name + "_fl")
    nc.vector.tensor_tensor(fl[:, :], xf[:, :], neg[:, :], OP.subtract)
    fr = pool.tile([R, x.shape[1]], F32, tag=name + "_fr")
    nc.vector.tensor_tensor(fr[:, :], d[:, :], neg[:, :], OP.add)
    return fl[:, :], fr[:, :]


def build_program(reps: int = 1):
    nc = bacc.Bacc("TRN2", target_bir_lowering=False, debug=False, num_swdge_queues=4)
    nc.dynamic_dma_scratch_size = 2 ** 16

    data = nc.dram_tensor("data_t", [NPX, CH], F32, kind="ExternalInput")
    rois_d = nc.dram_tensor("rois", [R, 5], F32, kind="ExternalInput")
    off_d = nc.dram_tensor("offs", [R, 2 * NB], F32, kind="ExternalInput")
    iopw_d = nc.dram_tensor("iota_pw", [R, NB], F32, kind="ExternalInput")
    ioph_d = nc.dram_tensor("iota_ph", [R, NB], F32, kind="ExternalInput")
    iden_d = nc.dram_tensor("identity", [R, R], F32, kind="ExternalInput")
    cmsk_d = nc.dram_tensor("cmasks", [128, NCH * NB], F32, kind="ExternalInput")
    out_d = nc.dram_tensor("out", [R, NB * CH], F32, kind="ExternalOutput")

    with tile.TileContext(nc) as tc:
        with (
            tc.tile_pool(name="const", bufs=1) as cst,
            tc.tile_pool(name="work", bufs=1) as wk,
            tc.tile_pool(name="gp", bufs=10) as gp,
            tc.tile_pool(name="gwp", bufs=12) as gwp,
            tc.tile_pool(name="obp", bufs=8) as obp,
            tc.tile_pool(name="psp", bufs=6, space="PSUM") as psp,
            tc.tile_pool(name="pst", bufs=2, space="PSUM") as pst,
        ):
            # ---- load inputs / constants to SBUF ----
            rois = cst.tile([R, 5], F32)
            nc.sync.dma_start(rois[:, :], rois_d.ap())
            off = cst.tile([R, 2 * NB], F32)
            nc.sync.dma_start(off[:, :], off_d.ap())
            iopw = cst.tile([R, NB], F32)
            nc.sync.dma_start(iopw[:, :], iopw_d.ap())
            ioph = cst.tile([R, NB], F32)
            nc.sync.dma_start(ioph[:, :], ioph_d.ap())
            iden = cst.tile([R, R], F32)
            nc.sync.dma_start(iden[:, :], iden_d.ap())
            cmsk = cst.tile([128, NCH * NB], F32)
            nc.sync.dma_start(cmsk[:, :], cmsk_d.ap())

            # ---- phase A: per-roi coordinate math, roi on partition ----
            # round(rois[:,1:5]) = floor(x + 0.5)
            rr = wk.tile([R, 4], F32)
            nc.vector.tensor_scalar(rr[:, :], rois[:, 1:5], 0.5, None, OP.add)
            rnd, _ = _floor(nc, wk, rr[:, :], "rnd")

            # start/end in feature coords
            swsh = wk.tile([R, 2], F32)
            nc.vector.tensor_scalar(swsh[:, :], rnd[:, 0:2], SCALE, -0.5, OP.mult, OP.add)
            eweh = wk.tile([R, 2], F32)
            nc.vector.tensor_scalar(
                eweh[:, :], rnd[:, 2:4], SCALE, SCALE - 0.5, OP.mult, OP.add
            )
            rwh0 = wk.tile([R, 2], F32)
            nc.vector.tensor_tensor(rwh0[:, :], eweh[:, :], swsh[:, :], OP.subtract)
            rwh = wk.tile([R, 2], F32)
            nc.vector.tensor_scalar(rwh[:, :], rwh0[:, :], 0.1, None, OP.max)
            bwh = wk.tile([R, 2], F32)
            nc.vector.tensor_scalar(bwh[:, :], rwh[:, :], 1.0 / P, None, OP.mult)
            swh = wk.tile([R, 2], F32)
            nc.vector.tensor_scalar(swh[:, :], bwh[:, :], 0.5, None, OP.mult)
            rwh01 = wk.tile([R, 2], F32)
            nc.vector.tensor_scalar(rwh01[:, :], rwh[:, :], TRANS_STD, None, OP.mult)
            ybase = wk.tile([R, 1], F32)
            nc.vector.tensor_scalar(ybase[:, :], rois[:, 0:1], float(H * W), None, OP.mult)

            # bin starts, shifted by learned offsets: [R, 49]
            def bin_start(iota, bcol, scol, tview, r01col, name):
                t0 = wk.tile([R, NB], F32, tag=name + "0")
                nc.vector.tensor_scalar(t0[:, :], iota, bcol, None, OP.mult)
                t1 = wk.tile([R, NB], F32, tag=name + "1")
                nc.vector.scalar_tensor_tensor(
                    t1[:, :], tview, r01col, t0[:, :], OP.mult, OP.add
                )
                t2 = wk.tile([R, NB], F32, tag=name + "2")
                nc.vector.tensor_scalar(t2[:, :], t1[:, :], scol, None, OP.add)
                return t2

            wstart = bin_start(
                iopw[:, :], bwh[:, 0:1], swsh[:, 0:1], off[:, 0:NB],
                rwh01[:, 0:1], "ws",
            )
            hstart = bin_start(
                ioph[:, :], bwh[:, 1:2], swsh[:, 1:2], off[:, NB : 2 * NB],
                rwh01[:, 1:2], "hs",
            )

            # sample positions [R, 98] = (bin, s)
            def samples(start, subcol, name):
                s2 = wk.tile([R, 2 * NB], F32, tag=name)
                v = s2[:, :].rearrange("p (b s) -> p b s", s=2)
                su = start[:, :].rearrange("p b -> p b", ).unsqueeze(2)
                nc.vector.tensor_copy(v[:, :, 0:1], su)
                nc.vector.tensor_scalar(v[:, :, 1:2], su, subcol, None, OP.add)
                return s2

            X2 = samples(wstart, swh[:, 0:1], "X2")
            Y2 = samples(hstart, swh[:, 1:2], "Y2")

            # per-axis: validity, clip, floor/frac, weight pairs, index pairs
            def axis_side(S2, lim, name):
                # valid = (S2 >= -0.5) & (S2 <= lim + 0.5)
                va = wk.tile([R, 2 * NB], F32, tag=name + "va")
                nc.vector.tensor_scalar(va[:, :], S2[:, :], -0.5, None, OP.is_ge)
                vv = wk.tile([R, 2 * NB], F32, tag=name + "vv")
                nc.vector.scalar_tensor_tensor(
                    vv[:, :], S2[:, :], lim + 0.5, va[:, :], OP.is_le, OP.mult
                )
                cl = wk.tile([R, 2 * NB], F32, tag=name + "cl")
                nc.vector.tensor_scalar(cl[:, :], S2[:, :], 0.0, lim, OP.max, OP.min)
                flo, fra = _floor(nc, wk, cl[:, :], name + "fl")
                # count over the 2 samples, per bin -> reciprocal (exact: 1 or .5)
                cnt = wk.tile([R, NB], F32, tag=name + "ct")
                vvv = vv[:, :].rearrange("p (b s) -> p b s", s=2)
                nc.vector.tensor_tensor(
                    cnt[:, :].unsqueeze(2),
                    vvv[:, :, 0:1], vvv[:, :, 1:2], OP.add,
                )
                eq2 = wk.tile([R, NB], F32, tag=name + "e2")
                nc.vector.tensor_scalar(eq2[:, :], cnt[:, :], 2.0, None, OP.is_equal)
                rc = wk.tile([R, NB], F32, tag=name + "rc")
                nc.vector.tensor_scalar(rc[:, :], eq2[:, :], -0.5, 1.0, OP.mult, OP.add)
                # weight pair: w0 = v*(1-f)*rc, w1 = v*f*rc  [R, 196] = (bin, s, c)
                rcb = rc[:, :].unsqueeze(2).broadcast_to([R, NB, 2])
                vr = wk.tile([R, 2 * NB], F32, tag=name + "vr")
                nc.vector.tensor_tensor(
                    vr[:, :].rearrange("p (b s) -> p b s", s=2), vvv, rcb, OP.mult
                )
                w1 = wk.tile([R, 2 * NB], F32, tag=name + "w1")
                nc.vector.tensor_tensor(w1[:, :], vr[:, :], fra, OP.mult)
                w0 = wk.tile([R, 2 * NB], F32, tag=name + "w0")
                nc.vector.tensor_tensor(w0[:, :], vr[:, :], w1[:, :], OP.subtract)
                W4 = wk.tile([R, 4 * NB], F32, tag=name + "W4")
                W4v = W4[:, :].rearrange("p (b s c) -> p b s c", s=2, c=2)
                w0v = w0[:, :].rearrange("p (b s) -> p b s", s=2).unsqueeze(3)
                w1v = w1[:, :].rearrange("p (b s) -> p b s", s=2).unsqueeze(3)
                nc.vector.tensor_copy(W4v[:, :, :, 0:1], w0v)
                nc.vector.tensor_copy(W4v[:, :, :, 1:2], w1v)
                # index pair: i0 = floor, i1 = min(floor+1, lim)
                I4 = wk.tile([R, 4 * NB], F32, tag=name + "I4")
                I4v = I4[:, :].rearrange("p (b s c) -> p b s c", s=2, c=2)
                flv = flo.rearrange("p (b s) -> p b s", s=2).unsqueeze(3)
                nc.vector.tensor_copy(I4v[:, :, :, 0:1], flv)
                nc.vector.tensor_scalar(I4v[:, :, :, 1:2], flv, 1.0, lim, OP.add, OP.min)
                return W4, I4

            WX4, XI4 = axis_side(X2, float(W - 1), "x")
            WY4, YI4 = axis_side(Y2, float(H - 1), "y")

            # y-side indices -> flat row base: b*H*W + y*W
            YIr = wk.tile([R, 4 * NB], F32)
            nc.vector.tensor_scalar(
                YIr[:, :], YI4[:, :], float(W), ybase[:, :], OP.mult, OP.add
            )

            # expand to full terms [R, 784] = (bin, sh, cy, sw, cx)
            Wt = wk.tile([R, T], F32)
            Wtv = Wt[:, :].rearrange("p (b h y s x) -> p b h y s x", h=2, y=2, s=2, x=2)
            IDX = wk.tile([R, T], F32)
            IDXv = IDX[:, :].rearrange(
                "p (b h y s x) -> p b h y s x", h=2, y=2, s=2, x=2
            )
            WY4v = WY4[:, :].rearrange("p (b h y) -> p b h y", h=2, y=2).unsqueeze(4).unsqueeze(5)
            YIrv = YIr[:, :].rearrange("p (b h y) -> p b h y", h=2, y=2).unsqueeze(4).unsqueeze(5)
            for k in range(4):
                s, x = k >> 1, k & 1
                nc.vector.tensor_copy(Wtv[:, :, :, :, s : s + 1, x : x + 1], WY4v)
                nc.vector.tensor_copy(IDXv[:, :, :, :, s : s + 1, x : x + 1], YIrv)
            WX4v = WX4[:, :].rearrange("p (b s x) -> p b s x", s=2, x=2).unsqueeze(2).unsqueeze(3)
            XI4v = XI4[:, :].rearrange("p (b s x) -> p b s x", s=2, x=2).unsqueeze(2).unsqueeze(3)
            for j in range(4):
                h, y = j >> 1, j & 1
                dstW = Wtv[:, :, h : h + 1, y : y + 1, :, :]
                dstI = IDXv[:, :, h : h + 1, y : y + 1, :, :]
                nc.vector.tensor_tensor(dstW, dstW, WX4v, OP.mult)
                nc.vector.tensor_tensor(dstI, dstI, XI4v, OP.add)
            # ---- phase B: transpose weights to [128, (n,c)]; build int16
            # gather indices in dma_gather's 16-lane-wrapped layout.
            # Descriptor i = m*16 + l reads IDXG[l, m]; lands at dest
            # partition i%128, col i//128.  With idx col m = n*56 + q*8 + rr
            # (rr<7 real, rr=7 pad), term (roi n, chunk q, bin-in-chunk rr,
            # corner k=l) lands at partition rr*16+k, dest col n*7+q.
            WT = wk.tile([128, R * NCH], F32)
            nc.vector.memset(WT[:, :], 0.0)
            WTv = WT[:, :].rearrange("p (n c) -> p c n", c=NCH)
            for c in range(NCH):
                kc = 128 if c < 6 else 16
                psA = pst.tile([kc, R], F32, tag="pstr")
                nc.tensor.transpose(
                    psA[:, :], Wt[:, c * 128 : c * 128 + kc], iden[:, :]
                )
                nc.vector.tensor_copy(
                    WTv[0:kc, c : c + 1, :], psA[:, :].unsqueeze(1)
                )
            IDXG = wk.tile([128, R * NB], mybir.dt.int16)
            nc.vector.memset(IDXG[:, :], 0)
            IDXGv = IDXG[:, :].rearrange("p (n m) -> p n m", m=NB)
            for b in range(NB):
                psB = pst.tile([16, R], F32, tag="pstr")
                nc.tensor.transpose(
                    psB[:, :], IDX[:, b * TPB : (b + 1) * TPB], iden[:, :]
                )
                nc.vector.tensor_copy(
                    IDXGv[0:16, :, b : b + 1], psB[:, :].unsqueeze(2)
                )
            # Q7 tx/rx cpus each read their own 16-partition window of the
            # index tensor -> replicate lane group 0 across all 8 groups.
            for grp in range(1, 8):
                nc.sync.dma_start(
                    IDXG[16 * grp : 16 * (grp + 1), :], IDXG[0:16, :]
                )

            # ---- phase C: gather + weighted reduce ----
            NI = T  # 784 descriptors per roi, no padding
            out_r = out_d.ap().rearrange("r (b c) -> r b c", c=CH)
            from contextlib import nullcontext
            loop_cm = tc.For_i(0, reps, 1) if reps > 1 else nullcontext()
            with loop_cm:
              for g in range(R // G):
                  gt = gp.tile([128, NCH * CH], F32)
                  # col 6 rows 16-127 are never gathered; clear for finite 0s
                  nc.vector.memset(gt[:, 6 * CH : 7 * CH], 0.0)
                  dest = gt[:, :].rearrange("p (j f) -> p j f", f=CH)
                  nc.gpsimd.dma_gather(
                      dest,
                      data.ap(),
                      IDXG[:, g * NB : (g + 1) * NB],
                      NI,
                      NI,
                      CH,
                      queue_num=g % 4,
                  )
                  for nl in range(G):
                      n = g * G + nl
                      # weighted mask for all 7 chunks of this roi in one op:
                      # wm[p, c, j] = cmask[p, c, j] * WT[p, n*7+c]
                      wm = gwp.tile([128, NCH * NB], F32)
                      wtb = (
                          WT[:, n * NCH : (n + 1) * NCH]
                          .unsqueeze(2)
                          .broadcast_to([128, NCH, NB])
                      )
                      nc.any.tensor_tensor(
                          wm[:, :].rearrange("p (c j) -> p c j", j=NB),
                          cmsk[:, :].rearrange("p (c j) -> p c j", j=NB),
                          wtb,
                          OP.mult,
                      )
                      ps = psp.tile([NB, CH], F32)
                      for c in range(NCH):
                          gv = gt[:, (nl * NCH + c) * CH : (nl * NCH + c + 1) * CH]
                          nc.tensor.matmul(
                              ps[:, :],
                              wm[:, c * NB : (c + 1) * NB],
                              gv,
                              start=(c == 0),
                              stop=(c == NCH - 1),
                          )
                      ob = obp.tile([NB, CH], F32)
                      nc.vector.tensor_copy(ob[:, :], ps[:, :])
                      nc.sync.dma_start(out_r[n : n + 1, :, :], ob[:, :])

    nc.finalize()
    return nc


def host_constants():
    iopw = np.tile((np.arange(NB) % P).astype(np.float32), (R, 1))
    ioph = np.tile((np.arange(NB) // P).astype(np.float32), (R, 1))
    iden = np.eye(R, dtype=np.float32)
    cm = np.zeros((128, NCH * NB), dtype=np.float32)
    for j in range(6):
        for p in range(128):
            cm[p, j * NB + 8 * j + p // TPB] = 1.0
    for p in range(TPB):
        cm[p, 6 * NB + 48] = 1.0
    return {"iota_pw": iopw, "iota_ph": ioph, "identity": iden, "cmasks": cm}


_cache = {}


def _program():
    if "nc" not in _cache:
        _cache["nc"] = build_program()
    return _cache["nc"]


def run(data, rois, offset, **spmd_kwargs):
    data = np.asarray(data, dtype=np.float32)
    rois = np.asarray(rois, dtype=np.float32)
    offset = np.asarray(offset, dtype=np.float32)
    n_rois = rois.shape[0]
    data_t = np.ascontiguousarray(data.transpose(0, 2, 3, 1)).reshape(NPX, CH)
    consts = host_constants()
    in_maps = []
    for c in range(N_CORES):
        sl = slice(c * R, (c + 1) * R)
        m = {
            "data_t": data_t,
            "rois": rois[sl],
            "offs": offset[sl].reshape(R, 2 * NB),
        }
        m.update(consts)
        in_maps.append(m)
    res = run_bass_kernel_spmd(
        _program(), in_maps, core_ids=list(range(N_CORES)), **spmd_kwargs
    )
    outs = np.concatenate([res.results[c]["out"] for c in range(N_CORES)], axis=0)
    out = outs.reshape(n_rois, NB, CH).transpose(0, 2, 1).reshape(n_rois, CH, P, P)
    return np.ascontiguousarray(out), res


def kernel(data, rois, offset):
    out, _ = run(data, rois, offset)
    return out



# revision 20
# speedup vs baseline: 1.7815x; 1.7815x over previous
"""Deformable PS-ROI pooling on Trainium2 (Bass/Tile), SPMD over 8 cores.

Strategy: data-parallel over ROIs (64 rois/core), feature map replicated in
DRAM in channel-last bf16 layout so each bilinear corner is one contiguous
512 B gather.  Per (roi, bin) the 4 samples x 4 corners = 16 gathered pixel
vectors land on 16 SBUF partitions.  Gathers are grouped 4 rois per call with
a column-interleaved descriptor layout (col = chunk*4 + roi-in-group, chunks
= 8 aligned bins = 128 descriptors) so every roi keeps an identical partition
pattern; bin 48 of the 4 rois packs into one shared column.  A block-diagonal
bf16 mask matmul on the PE reduces each chunk into [8, 256] PSUM rows, with
all bilinear / validity / 1-over-count factors pre-folded into a per-partition
scalar weight (masks precomputed for all rois outside the loop).
"""

import numpy as np
import ml_dtypes

import concourse.bass as bass
import concourse.bacc as bacc
import concourse.mybir as mybir
from concourse import tile
from concourse.bass_utils import run_bass_kernel_spmd

F32 = mybir.dt.float32
BF16 = mybir.dt.bfloat16
I32 = mybir.dt.int32
I16 = mybir.dt.int16
OP = mybir.AluOpType

N_CORES = 8
R = 64                  # rois per core
P = 7                   # pooled output size
NB = P * P              # 49 bins
CH = 256                # channels
H = W = 128             # feature map spatial
B = 2                   # batch
NPX = B * H * W         # 32768 flat pixels
TPB = 16                # terms (sample x corner) per bin
T = NB * TPB            # 784 terms per roi
G = 4                   # rois per gather group
NG = R // G             # 16 groups
NCK = 6                 # full 8-bin chunks per roi (bins 0..47)
NCOL = NCK * G + 1      # 25 gather dest columns per group (last = bin48+pad)
NI = NCOL * 128         # 3200 descriptors per gather call
MW = NI // 16           # 200 idx columns per group
SCALE = 0.0625
TRANS_STD = 0.1


def _floor(nc, pool, x, name):
    """floor(x) robust to convert rounding mode: returns (floor_f32, frac)."""
    xi = pool.tile([R, x.shape[1]], I32, tag=name + "_i")
    nc.vector.tensor_copy(xi[:, :], x)
    xf = pool.tile([R, x.shape[1]], F32, tag=name + "_f")
    nc.vector.tensor_copy(xf[:, :], xi[:, :])
    d = pool.tile([R, x.shape[1]], F32, tag=name + "_d")
    nc.vector.tensor_tensor(d[:, :], x, xf[:, :], OP.subtract)
    neg = pool.tile([R, x.shape[1]], F32, tag=name + "_n")
    nc.vector.tensor_scalar(neg[:, :], d[:, :], 0.0, None, OP.is_lt)
    fl = pool.tile([R, x.shape[1]], F32, tag=name + "_fl")
    nc.vector.tensor_tensor(fl[:, :], xf[:, :], neg[:, :], OP.subtract)
    fr = pool.tile([R, x.shape[1]], F32, tag=name + "_fr")
    nc.vector.tensor_tensor(fr[:, :], d[:, :], neg[:, :], OP.add)
    return fl[:, :], fr[:, :]


def build_program(reps: int = 1, unroll: int = 1):
    nc = bacc.Bacc("TRN2", target_bir_lowering=False, debug=False, num_swdge_queues=4)
    nc.dynamic_dma_scratch_size = 2 ** 16

    data = nc.dram_tensor("data_t", [NPX, CH], BF16, kind="ExternalInput")
    rois_d = nc.dram_tensor("rois", [R, 5], F32, kind="ExternalInput")
    off_d = nc.dram_tensor("offs", [R, 2 * NB], F32, kind="ExternalInput")
    iopw_d = nc.dram_tensor("iota_pw", [R, NB], F32, kind="ExternalInput")
    ioph_d = nc.dram_tensor("iota_ph", [R, NB], F32, kind="ExternalInput")
    iden_d = nc.dram_tensor("identity", [R, R], F32, kind="ExternalInput")
    cm8_d = nc.dram_tensor("cm8", [128, NCK * 48], F32, kind="ExternalInput")
    out_d = nc.dram_tensor("out", [R, NB * CH], BF16, kind="ExternalOutput")

    with tile.TileContext(nc) as tc:
        with (
            tc.tile_pool(name="const", bufs=1) as cst,
            tc.tile_pool(name="work", bufs=1) as wk,
            tc.tile_pool(name="gp", bufs=4) as gp,
            tc.tile_pool(name="obp", bufs=8) as obp,
            tc.tile_pool(name="psp", bufs=4, space="PSUM") as psp,
            tc.tile_pool(name="pss", bufs=2, space="PSUM") as pss,
            tc.tile_pool(name="pst", bufs=2, space="PSUM") as pst,
        ):
            # ---- load inputs / constants to SBUF ----
            rois = cst.tile([R, 5], F32)
            nc.sync.dma_start(rois[:, :], rois_d.ap())
            off = cst.tile([R, 2 * NB], F32)
            nc.sync.dma_start(off[:, :], off_d.ap())
            iopw = cst.tile([R, NB], F32)
            nc.sync.dma_start(iopw[:, :], iopw_d.ap())
            ioph = cst.tile([R, NB], F32)
            nc.sync.dma_start(ioph[:, :], ioph_d.ap())
            iden = cst.tile([R, R], F32)
            nc.sync.dma_start(iden[:, :], iden_d.ap())
            cm8 = cst.tile([128, NCK * 48], F32)
            nc.sync.dma_start(cm8[:, :], cm8_d.ap())

            # ---- phase A: per-roi coordinate math, roi on partition ----
            # round(rois[:,1:5]) = floor(x + 0.5)
            rr = wk.tile([R, 4], F32)
            nc.vector.tensor_scalar(rr[:, :], rois[:, 1:5], 0.5, None, OP.add)
            rnd, _ = _floor(nc, wk, rr[:, :], "rnd")

            # start/end in feature coords
            swsh = wk.tile([R, 2], F32)
            nc.vector.tensor_scalar(swsh[:, :], rnd[:, 0:2], SCALE, -0.5, OP.mult, OP.add)
            eweh = wk.tile([R, 2], F32)
            nc.vector.tensor_scalar(
                eweh[:, :], rnd[:, 2:4], SCALE, SCALE - 0.5, OP.mult, OP.add
            )
            rwh0 = wk.tile([R, 2], F32)
            nc.vector.tensor_tensor(rwh0[:, :], eweh[:, :], swsh[:, :], OP.subtract)
            rwh = wk.tile([R, 2], F32)
            nc.vector.tensor_scalar(rwh[:, :], rwh0[:, :], 0.1, None, OP.max)
            bwh = wk.tile([R, 2], F32)
            nc.vector.tensor_scalar(bwh[:, :], rwh[:, :], 1.0 / P, None, OP.mult)
            swh = wk.tile([R, 2], F32)
            nc.vector.tensor_scalar(swh[:, :], bwh[:, :], 0.5, None, OP.mult)
            rwh01 = wk.tile([R, 2], F32)
            nc.vector.tensor_scalar(rwh01[:, :], rwh[:, :], TRANS_STD, None, OP.mult)
            ybase = wk.tile([R, 1], F32)
            nc.vector.tensor_scalar(ybase[:, :], rois[:, 0:1], float(H * W), None, OP.mult)

            # bin starts, shifted by learned offsets: [R, 49]
            def bin_start(iota, bcol, scol, tview, r01col, name):
                t0 = wk.tile([R, NB], F32, tag=name + "0")
                nc.vector.tensor_scalar(t0[:, :], iota, bcol, None, OP.mult)
                t1 = wk.tile([R, NB], F32, tag=name + "1")
                nc.vector.scalar_tensor_tensor(
                    t1[:, :], tview, r01col, t0[:, :], OP.mult, OP.add
                )
                t2 = wk.tile([R, NB], F32, tag=name + "2")
                nc.vector.tensor_scalar(t2[:, :], t1[:, :], scol, None, OP.add)
                return t2

            wstart = bin_start(
                iopw[:, :], bwh[:, 0:1], swsh[:, 0:1], off[:, 0:NB],
                rwh01[:, 0:1], "ws",
            )
            hstart = bin_start(
                ioph[:, :], bwh[:, 1:2], swsh[:, 1:2], off[:, NB : 2 * NB],
                rwh01[:, 1:2], "hs",
            )

            # sample positions [R, 98] = (bin, s)
            def samples(start, subcol, name):
                s2 = wk.tile([R, 2 * NB], F32, tag=name)
                v = s2[:, :].rearrange("p (b s) -> p b s", s=2)
                su = start[:, :].rearrange("p b -> p b", ).unsqueeze(2)
                nc.vector.tensor_copy(v[:, :, 0:1], su)
                nc.vector.tensor_scalar(v[:, :, 1:2], su, subcol, None, OP.add)
                return s2

            X2 = samples(wstart, swh[:, 0:1], "X2")
            Y2 = samples(hstart, swh[:, 1:2], "Y2")

            # per-axis: validity, clip, floor/frac, weight pairs, index pairs
            def axis_side(S2, lim, name):
                # valid = (S2 >= -0.5) & (S2 <= lim + 0.5)
                va = wk.tile([R, 2 * NB], F32, tag=name + "va")
                nc.vector.tensor_scalar(va[:, :], S2[:, :], -0.5, None, OP.is_ge)
                vv = wk.tile([R, 2 * NB], F32, tag=name + "vv")
                nc.vector.scalar_tensor_tensor(
                    vv[:, :], S2[:, :], lim + 0.5, va[:, :], OP.is_le, OP.mult
                )
                cl = wk.tile([R, 2 * NB], F32, tag=name + "cl")
                nc.vector.tensor_scalar(cl[:, :], S2[:, :], 0.0, lim, OP.max, OP.min)
                flo, fra = _floor(nc, wk, cl[:, :], name + "fl")
                # count over the 2 samples, per bin -> reciprocal (exact: 1 or .5)
                cnt = wk.tile([R, NB], F32, tag=name + "ct")
                vvv = vv[:, :].rearrange("p (b s) -> p b s", s=2)
                nc.vector.tensor_tensor(
                    cnt[:, :].unsqueeze(2),
                    vvv[:, :, 0:1], vvv[:, :, 1:2], OP.add,
                )
                eq2 = wk.tile([R, NB], F32, tag=name + "e2")
                nc.vector.tensor_scalar(eq2[:, :], cnt[:, :], 2.0, None, OP.is_equal)
                rc = wk.tile([R, NB], F32, tag=name + "rc")
                nc.vector.tensor_scalar(rc[:, :], eq2[:, :], -0.5, 1.0, OP.mult, OP.add)
                # weight pair: w0 = v*(1-f)*rc, w1 = v*f*rc  [R, 196] = (bin, s, c)
                rcb = rc[:, :].unsqueeze(2).broadcast_to([R, NB, 2])
                vr = wk.tile([R, 2 * NB], F32, tag=name + "vr")
                nc.vector.tensor_tensor(
                    vr[:, :].rearrange("p (b s) -> p b s", s=2), vvv, rcb, OP.mult
                )
                w1 = wk.tile([R, 2 * NB], F32, tag=name + "w1")
                nc.vector.tensor_tensor(w1[:, :], vr[:, :], fra, OP.mult)
                w0 = wk.tile([R, 2 * NB], F32, tag=name + "w0")
                nc.vector.tensor_tensor(w0[:, :], vr[:, :], w1[:, :], OP.subtract)
                W4 = wk.tile([R, 4 * NB], F32, tag=name + "W4")
                W4v = W4[:, :].rearrange("p (b s c) -> p b s c", s=2, c=2)
                w0v = w0[:, :].rearrange("p (b s) -> p b s", s=2).unsqueeze(3)
                w1v = w1[:, :].rearrange("p (b s) -> p b s", s=2).unsqueeze(3)
                nc.vector.tensor_copy(W4v[:, :, :, 0:1], w0v)
                nc.vector.tensor_copy(W4v[:, :, :, 1:2], w1v)
                # index pair: i0 = floor, i1 = min(floor+1, lim)
                I4 = wk.tile([R, 4 * NB], F32, tag=name + "I4")
                I4v = I4[:, :].rearrange("p (b s c) -> p b s c", s=2, c=2)
                flv = flo.rearrange("p (b s) -> p b s", s=2).unsqueeze(3)
                nc.vector.tensor_copy(I4v[:, :, :, 0:1], flv)
                nc.vector.tensor_scalar(I4v[:, :, :, 1:2], flv, 1.0, lim, OP.add, OP.min)
                return W4, I4

            WX4, XI4 = axis_side(X2, float(W - 1), "x")
            WY4, YI4 = axis_side(Y2, float(H - 1), "y")

            # y-side indices -> flat row base: b*H*W + y*W
            YIr = wk.tile([R, 4 * NB], F32)
            nc.vector.tensor_scalar(
                YIr[:, :], YI4[:, :], float(W), ybase[:, :], OP.mult, OP.add
            )

            # expand to full terms [R, 784] = (bin, sh, cy, sw, cx)
            Wt = wk.tile([R, T], F32)
            Wtv = Wt[:, :].rearrange("p (b h y s x) -> p b h y s x", h=2, y=2, s=2, x=2)
            IDX = wk.tile([R, T], F32)
            IDXv = IDX[:, :].rearrange(
                "p (b h y s x) -> p b h y s x", h=2, y=2, s=2, x=2
            )
            WY4v = WY4[:, :].rearrange("p (b h y) -> p b h y", h=2, y=2).unsqueeze(4).unsqueeze(5)
            YIrv = YIr[:, :].rearrange("p (b h y) -> p b h y", h=2, y=2).unsqueeze(4).unsqueeze(5)
            for k in range(4):
                s, x = k >> 1, k & 1
                nc.vector.tensor_copy(Wtv[:, :, :, :, s : s + 1, x : x + 1], WY4v)
                nc.vector.tensor_copy(IDXv[:, :, :, :, s : s + 1, x : x + 1], YIrv)
            WX4v = WX4[:, :].rearrange("p (b s x) -> p b s x", s=2, x=2).unsqueeze(2).unsqueeze(3)
            XI4v = XI4[:, :].rearrange("p (b s x) -> p b s x", s=2, x=2).unsqueeze(2).unsqueeze(3)
            for j in range(4):
                h, y = j >> 1, j & 1
                dstW = Wtv[:, :, h : h + 1, y : y + 1, :, :]
                dstI = IDXv[:, :, h : h + 1, y : y + 1, :, :]
                nc.vector.tensor_tensor(dstW, dstW, WX4v, OP.mult)
                nc.vector.tensor_tensor(dstI, dstI, XI4v, OP.add)

            # ---- phase B: transpose weights to [128, (n, c)]; fold into
            # per-roi bf16 block-diag masks; build int16 gather indices in
            # dma_gather's 16-lane-wrapped layout.
            #
            # Per 4-roi group the descriptor slot i = col*128 + 16q + l with
            # col = 4c + k (chunk c in 0..5, roi-in-group k) covers bin 8c+q
            # term l of roi 4g+k; col 24 packs bin48 of the 4 rois at
            # partitions 16k+l (trailing 64 slots pad with idx 0).  Idx entry
            # for desc i lives at IDXG[i%16, g*200 + i//16].
            WT = wk.tile([128, R * NCK], F32)
            WTv = WT[:, :].rearrange("p (n c) -> p n c", c=NCK)
            for c in range(NCK):
                psA = pst.tile([128, R], F32, tag="pstr")
                nc.tensor.transpose(psA[:, :], Wt[:, c * 128 : (c + 1) * 128], iden[:, :])
                nc.vector.tensor_copy(WTv[:, :, c : c + 1], psA[:, :].unsqueeze(2))
            # bin48 weights: wt48[16k+l, n] = Wt[n, 768+l] for k = n%4
            psAw = pst.tile([16, R], F32, tag="pstr")
            nc.tensor.transpose(psAw[:, :], Wt[:, 768:784], iden[:, :])
            wt48 = wk.tile([128, R], BF16)
            nc.vector.memset(wt48[:, :], 0.0)
            aw = wk.tile([16, R], BF16)
            nc.vector.tensor_copy(aw[:, :], psAw[:, :])
            awv = aw[:, :].rearrange("p (g k) -> p g k", k=G)
            for k in range(G):
                dst = wt48[16 * k : 16 * (k + 1), :].rearrange(
                    "p (g k) -> p g k", k=G
                )
                nc.sync.dma_start(dst[:, :, k : k + 1], awv[:, :, k : k + 1])
            # all-roi masks: wma[p, n, c, j] = WT[p, n, c] * (j == 8c + p//16)
            # (full 48-bin mask per chunk; 6 chunk matmuls accumulate into one
            # [48, 256] PSUM region since PE outs must start at partition 0)
            wma = wk.tile([128, R * NCK * 48], BF16)
            wmav = wma[:, :].rearrange("p (n c j) -> p n c j", c=NCK, j=48)
            nc.any.tensor_tensor(
                wmav,
                WTv.unsqueeze(3).broadcast_to([128, R, NCK, 48]),
                cm8[:, :].rearrange("p (c j) -> p c j", j=48)
                .unsqueeze(1)
                .broadcast_to([128, R, NCK, 48]),
                OP.mult,
            )

            IDXG = wk.tile([128, NG * MW], I16)
            IDXGv = IDXG[:, :].rearrange("p (g w q) -> p g w q", w=NCOL, q=8)
            for b in range(48):
                c, q = b // 8, b % 8
                psB = pst.tile([16, R], F32, tag="pstr")
                nc.tensor.transpose(psB[:, :], IDX[:, b * TPB : (b + 1) * TPB], iden[:, :])
                src = psB[:, :].rearrange("p (g k) -> p g k", k=G).unsqueeze(3)
                nc.vector.tensor_copy(
                    IDXGv[0:16, :, G * c : G * (c + 1), q : q + 1]
                    .rearrange("p g k q -> p g k q"),
                    src,
                )
            psB48 = pst.tile([16, R], F32, tag="pstr")
            nc.tensor.transpose(psB48[:, :], IDX[:, 768:784], iden[:, :])
            nc.vector.tensor_copy(
                IDXGv[0:16, :, NCOL - 1 : NCOL, 0:G],
                psB48[:, :].rearrange("p (g k) -> p g k", k=G).unsqueeze(2),
            )
            nc.vector.memset(IDXGv[0:16, :, NCOL - 1 : NCOL, G:8], 0)
            # Q7 tx/rx cpus each read their own 16-partition window of the
            # index tensor -> replicate lane group 0 across all 8 groups.
            for grp in range(1, 8):
                nc.sync.dma_start(
                    IDXG[16 * grp : 16 * (grp + 1), :], IDXG[0:16, :]
                )

            # ---- phase C: gather + weighted reduce ----
            out_r = out_d.ap().rearrange("r (b c) -> r b c", c=CH)
            # bin48 staging: stage[k, g, :] = bin48 of roi 4g+k (one DMA/pass)
            stage = wk.tile([G, NG * CH], BF16)
            stagev = stage[:, :].rearrange("p (g c) -> p g c", c=CH)
            out48_r = out_d.ap().rearrange(
                "(g k) (b c) -> k g b c", k=G, c=CH
            )[:, :, NB - 1, :]
            from contextlib import nullcontext
            loop_cm = tc.For_i(0, reps, 1) if reps > 1 else nullcontext()
            with loop_cm:
              for g in range(NG * unroll):
                  g = g % NG
                  gt = gp.tile([128, NCOL * CH], BF16)
                  dest = gt[:, :].rearrange("p (j f) -> p j f", f=CH)
                  for sub, (c0, c1) in enumerate([(0, 8), (8, 16), (16, 24), (24, 25)]):
                      nsub = (c1 - c0) * 128
                      nc.gpsimd.dma_gather(
                          dest[:, c0:c1, :],
                          data.ap(),
                          IDXG[:, g * MW + c0 * 8 : g * MW + c1 * 8],
                          nsub,
                          nsub,
                          CH,
                          queue_num=sub,
                      )
                  gv = gt[:, :].rearrange("p (j f) -> p j f", f=CH)
                  # bin48 of the 4 rois via one shared matmul
                  ps48 = pss.tile([G, CH], F32)
                  nc.tensor.matmul(
                      ps48[:, :], wt48[:, g * G : (g + 1) * G],
                      gv[:, NCOL - 1, :], start=True, stop=True,
                  )
                  nc.vector.tensor_copy(stagev[:, g : g + 1, :], ps48[:, :].unsqueeze(1))
                  ob4 = obp.tile([48, G * CH], BF16)
                  ob4v = ob4[:, :].rearrange("p (k c) -> p k c", c=CH)
                  for k in range(G):
                      n = g * G + k
                      ps = psp.tile([48, CH], F32)
                      for c in range(NCK):
                          nc.tensor.matmul(
                              ps[:, :],
                              wmav[:, n, c, :],
                              gv[:, G * c + k, :],
                              start=(c == 0), stop=(c == NCK - 1),
                          )
                      if k % 2 == 0:
                          nc.vector.tensor_copy(ob4v[:, k, :], ps[:, :])
                      else:
                          nc.scalar.activation(
                              ob4v[:, k, :], ps[:, :],
                              mybir.ActivationFunctionType.Copy,
                          )
                  nc.sync.dma_start(
                      out_r[g * G : (g + 1) * G, 0:48, :].rearrange(
                          "r b c -> b r c"
                      ),
                      ob4[:, :],
                  )
              nc.sync.dma_start(out48_r, stagev)

    nc.finalize()
    return nc


def host_constants():
    iopw = np.tile((np.arange(NB) % P).astype(np.float32), (R, 1))
    ioph = np.tile((np.arange(NB) // P).astype(np.float32), (R, 1))
    iden = np.eye(R, dtype=np.float32)
    cm8 = np.zeros((128, NCK * 48), dtype=np.float32)
    for c in range(NCK):
        for p in range(128):
            cm8[p, c * 48 + 8 * c + p // 16] = 1.0
    return {"iota_pw": iopw, "iota_ph": ioph, "identity": iden, "cm8": cm8}


_cache = {}


def _program():
    if "nc" not in _cache:
        _cache["nc"] = build_program()
    return _cache["nc"]


def make_in_maps(data, rois, offset):
    data = np.asarray(data, dtype=np.float32)
    rois = np.asarray(rois, dtype=np.float32)
    offset = np.asarray(offset, dtype=np.float32)
    data_t = (
        np.ascontiguousarray(data.transpose(0, 2, 3, 1))
        .reshape(NPX, CH)
        .astype(ml_dtypes.bfloat16)
    )
    consts = host_constants()
    in_maps = []
    for c in range(N_CORES):
        sl = slice(c * R, (c + 1) * R)
        m = {
            "data_t": data_t,
            "rois": rois[sl],
            "offs": offset[sl].reshape(R, 2 * NB),
        }
        m.update(consts)
        in_maps.append(m)
    return in_maps


def run(data, rois, offset, **spmd_kwargs):
    n_rois = np.asarray(rois).shape[0]
    in_maps = make_in_maps(data, rois, offset)
    res = run_bass_kernel_spmd(
        _program(), in_maps, core_ids=list(range(N_CORES)), **spmd_kwargs
    )
    outs = np.concatenate([res.results[c]["out"] for c in range(N_CORES)], axis=0)
    out = (
        outs.astype(np.float32)
        .reshape(n_rois, NB, CH)
        .transpose(0, 2, 1)
        .reshape(n_rois, CH, P, P)
    )
    return np.ascontiguousarray(out), res


def kernel(data, rois, offset):
    out, _ = run(data, rois, offset)
    return out


# revision 21
# speedup vs baseline: 1.9034x; 1.0684x over previous
"""Deformable PS-ROI pooling on Trainium2 (Bass/Tile), SPMD over 8 cores.

Strategy: data-parallel over ROIs (64 rois/core), feature map replicated in
DRAM in channel-last bf16 layout so each bilinear corner is one contiguous
512 B gather.  Per (roi, bin) the 4 samples x 4 corners = 16 gathered pixel
vectors land on 16 SBUF partitions.  Gathers are grouped 4 rois per 25-column
descriptor block with a column-interleaved layout (col = chunk*4 +
roi-in-group, chunk = 8 aligned bins = 128 descriptors) so every roi keeps an
identical partition pattern; bin 48 of the 4 rois packs into the shared 25th
column.  Each block is issued as 4 dma_gather sub-calls of <= 1024
descriptors (a hard Q7 ucode cap) on SWDGE queues 0-3.  Six accumulating
bf16 block-diagonal mask matmuls reduce each roi into a [48, 256] PSUM tile
(plus one shared bin48 matmul per group), with all bilinear / validity /
1-over-count factors pre-folded into a per-partition scalar weight; masks for
all rois are precomputed outside the loop.  Outputs are written bf16, one
batched DMA per group plus a per-pass bin48 staging DMA.
"""

import numpy as np
import ml_dtypes

import concourse.bass as bass
import concourse.bacc as bacc
import concourse.mybir as mybir
from concourse import tile
from concourse.bass_utils import run_bass_kernel_spmd

F32 = mybir.dt.float32
BF16 = mybir.dt.bfloat16
I32 = mybir.dt.int32
I16 = mybir.dt.int16
OP = mybir.AluOpType

N_CORES = 8
R = 64                  # rois per core
P = 7                   # pooled output size
NB = P * P              # 49 bins
CH = 256                # channels
H = W = 128             # feature map spatial
B = 2                   # batch
NPX = B * H * W         # 32768 flat pixels
TPB = 16                # terms (sample x corner) per bin
T = NB * TPB            # 784 terms per roi
G = 4                   # rois per gather group
NG = R // G             # 16 groups
NCK = 6                 # full 8-bin chunks per roi (bins 0..47)
NCOL = NCK * G + 1      # 25 gather dest columns per group (last = bin48+pad)
NI = NCOL * 128         # 3200 descriptors per gather call
MW = NI // 16           # 200 idx columns per group
SCALE = 0.0625
TRANS_STD = 0.1


def _floor(nc, pool, x, name):
    """floor(x) robust to convert rounding mode: returns (floor_f32, frac)."""
    xi = pool.tile([R, x.shape[1]], I32, tag=name + "_i")
    nc.vector.tensor_copy(xi[:, :], x)
    xf = pool.tile([R, x.shape[1]], F32, tag=name + "_f")
    nc.vector.tensor_copy(xf[:, :], xi[:, :])
    d = pool.tile([R, x.shape[1]], F32, tag=name + "_d")
    nc.vector.tensor_tensor(d[:, :], x, xf[:, :], OP.subtract)
    neg = pool.tile([R, x.shape[1]], F32, tag=name + "_n")
    nc.vector.tensor_scalar(neg[:, :], d[:, :], 0.0, None, OP.is_lt)
    fl = pool.tile([R, x.shape[1]], F32, tag=name + "_fl")
    nc.vector.tensor_tensor(fl[:, :], xf[:, :], neg[:, :], OP.subtract)
    fr = pool.tile([R, x.shape[1]], F32, tag=name + "_fr")
    nc.vector.tensor_tensor(fr[:, :], d[:, :], neg[:, :], OP.add)
    return fl[:, :], fr[:, :]


def build_program(reps: int = 1, unroll: int = 1):
    nc = bacc.Bacc("TRN2", target_bir_lowering=False, debug=False, num_swdge_queues=4)
    nc.dynamic_dma_scratch_size = 2 ** 16

    data = nc.dram_tensor("data_t", [NPX, CH], BF16, kind="ExternalInput")
    rois_d = nc.dram_tensor("rois", [R, 5], F32, kind="ExternalInput")
    off_d = nc.dram_tensor("offs", [R, 2 * NB], F32, kind="ExternalInput")
    iopw_d = nc.dram_tensor("iota_pw", [R, NB], F32, kind="ExternalInput")
    ioph_d = nc.dram_tensor("iota_ph", [R, NB], F32, kind="ExternalInput")
    iden_d = nc.dram_tensor("identity", [R, R], F32, kind="ExternalInput")
    cm8_d = nc.dram_tensor("cm8", [128, NCK * 48], F32, kind="ExternalInput")
    out_d = nc.dram_tensor("out", [R, NB * CH], BF16, kind="ExternalOutput")

    with tile.TileContext(nc) as tc:
        with (
            tc.tile_pool(name="const", bufs=1) as cst,
            tc.tile_pool(name="work", bufs=1) as wk,
            tc.tile_pool(name="gp", bufs=4) as gp,
            tc.tile_pool(name="obp", bufs=8) as obp,
            tc.tile_pool(name="psp", bufs=4, space="PSUM") as psp,
            tc.tile_pool(name="pss", bufs=2, space="PSUM") as pss,
            tc.tile_pool(name="pst", bufs=2, space="PSUM") as pst,
        ):
            # ---- load inputs / constants to SBUF ----
            rois = cst.tile([R, 5], F32)
            nc.sync.dma_start(rois[:, :], rois_d.ap())
            off = cst.tile([R, 2 * NB], F32)
            nc.sync.dma_start(off[:, :], off_d.ap())
            iopw = cst.tile([R, NB], F32)
            nc.sync.dma_start(iopw[:, :], iopw_d.ap())
            ioph = cst.tile([R, NB], F32)
            nc.sync.dma_start(ioph[:, :], ioph_d.ap())
            iden = cst.tile([R, R], F32)
            nc.sync.dma_start(iden[:, :], iden_d.ap())
            cm8 = cst.tile([128, NCK * 48], F32)
            nc.sync.dma_start(cm8[:, :], cm8_d.ap())

            # ---- phase A: per-roi coordinate math, roi on partition ----
            # round(rois[:,1:5]) = floor(x + 0.5)
            rr = wk.tile([R, 4], F32)
            nc.vector.tensor_scalar(rr[:, :], rois[:, 1:5], 0.5, None, OP.add)
            rnd, _ = _floor(nc, wk, rr[:, :], "rnd")

            # start/end in feature coords
            swsh = wk.tile([R, 2], F32)
            nc.vector.tensor_scalar(swsh[:, :], rnd[:, 0:2], SCALE, -0.5, OP.mult, OP.add)
            eweh = wk.tile([R, 2], F32)
            nc.vector.tensor_scalar(
                eweh[:, :], rnd[:, 2:4], SCALE, SCALE - 0.5, OP.mult, OP.add
            )
            rwh0 = wk.tile([R, 2], F32)
            nc.vector.tensor_tensor(rwh0[:, :], eweh[:, :], swsh[:, :], OP.subtract)
            rwh = wk.tile([R, 2], F32)
            nc.vector.tensor_scalar(rwh[:, :], rwh0[:, :], 0.1, None, OP.max)
            bwh = wk.tile([R, 2], F32)
            nc.vector.tensor_scalar(bwh[:, :], rwh[:, :], 1.0 / P, None, OP.mult)
            swh = wk.tile([R, 2], F32)
            nc.vector.tensor_scalar(swh[:, :], bwh[:, :], 0.5, None, OP.mult)
            rwh01 = wk.tile([R, 2], F32)
            nc.vector.tensor_scalar(rwh01[:, :], rwh[:, :], TRANS_STD, None, OP.mult)
            ybase = wk.tile([R, 1], F32)
            nc.vector.tensor_scalar(ybase[:, :], rois[:, 0:1], float(H * W), None, OP.mult)

            # bin starts, shifted by learned offsets: [R, 49]
            def bin_start(iota, bcol, scol, tview, r01col, name):
                t0 = wk.tile([R, NB], F32, tag=name + "0")
                nc.vector.tensor_scalar(t0[:, :], iota, bcol, None, OP.mult)
                t1 = wk.tile([R, NB], F32, tag=name + "1")
                nc.vector.scalar_tensor_tensor(
                    t1[:, :], tview, r01col, t0[:, :], OP.mult, OP.add
                )
                t2 = wk.tile([R, NB], F32, tag=name + "2")
                nc.vector.tensor_scalar(t2[:, :], t1[:, :], scol, None, OP.add)
                return t2

            wstart = bin_start(
                iopw[:, :], bwh[:, 0:1], swsh[:, 0:1], off[:, 0:NB],
                rwh01[:, 0:1], "ws",
            )
            hstart = bin_start(
                ioph[:, :], bwh[:, 1:2], swsh[:, 1:2], off[:, NB : 2 * NB],
                rwh01[:, 1:2], "hs",
            )

            # sample positions [R, 98] = (bin, s)
            def samples(start, subcol, name):
                s2 = wk.tile([R, 2 * NB], F32, tag=name)
                v = s2[:, :].rearrange("p (b s) -> p b s", s=2)
                su = start[:, :].rearrange("p b -> p b", ).unsqueeze(2)
                nc.vector.tensor_copy(v[:, :, 0:1], su)
                nc.vector.tensor_scalar(v[:, :, 1:2], su, subcol, None, OP.add)
                return s2

            X2 = samples(wstart, swh[:, 0:1], "X2")
            Y2 = samples(hstart, swh[:, 1:2], "Y2")

            # per-axis: validity, clip, floor/frac, weight pairs, index pairs
            def axis_side(S2, lim, name):
                # valid = (S2 >= -0.5) & (S2 <= lim + 0.5)
                va = wk.tile([R, 2 * NB], F32, tag=name + "va")
                nc.vector.tensor_scalar(va[:, :], S2[:, :], -0.5, None, OP.is_ge)
                vv = wk.tile([R, 2 * NB], F32, tag=name + "vv")
                nc.vector.scalar_tensor_tensor(
                    vv[:, :], S2[:, :], lim + 0.5, va[:, :], OP.is_le, OP.mult
                )
                cl = wk.tile([R, 2 * NB], F32, tag=name + "cl")
                nc.vector.tensor_scalar(cl[:, :], S2[:, :], 0.0, lim, OP.max, OP.min)
                flo, fra = _floor(nc, wk, cl[:, :], name + "fl")
                # count over the 2 samples, per bin -> reciprocal (exact: 1 or .5)
                cnt = wk.tile([R, NB], F32, tag=name + "ct")
                vvv = vv[:, :].rearrange("p (b s) -> p b s", s=2)
                nc.vector.tensor_tensor(
                    cnt[:, :].unsqueeze(2),
                    vvv[:, :, 0:1], vvv[:, :, 1:2], OP.add,
                )
                eq2 = wk.tile([R, NB], F32, tag=name + "e2")
                nc.vector.tensor_scalar(eq2[:, :], cnt[:, :], 2.0, None, OP.is_equal)
                rc = wk.tile([R, NB], F32, tag=name + "rc")
                nc.vector.tensor_scalar(rc[:, :], eq2[:, :], -0.5, 1.0, OP.mult, OP.add)
                # weight pair: w0 = v*(1-f)*rc, w1 = v*f*rc  [R, 196] = (bin, s, c)
                rcb = rc[:, :].unsqueeze(2).broadcast_to([R, NB, 2])
                vr = wk.tile([R, 2 * NB], F32, tag=name + "vr")
                nc.vector.tensor_tensor(
                    vr[:, :].rearrange("p (b s) -> p b s", s=2), vvv, rcb, OP.mult
                )
                w1 = wk.tile([R, 2 * NB], F32, tag=name + "w1")
                nc.vector.tensor_tensor(w1[:, :], vr[:, :], fra, OP.mult)
                w0 = wk.tile([R, 2 * NB], F32, tag=name + "w0")
                nc.vector.tensor_tensor(w0[:, :], vr[:, :], w1[:, :], OP.subtract)
                W4 = wk.tile([R, 4 * NB], F32, tag=name + "W4")
                W4v = W4[:, :].rearrange("p (b s c) -> p b s c", s=2, c=2)
                w0v = w0[:, :].rearrange("p (b s) -> p b s", s=2).unsqueeze(3)
                w1v = w1[:, :].rearrange("p (b s) -> p b s", s=2).unsqueeze(3)
                nc.vector.tensor_copy(W4v[:, :, :, 0:1], w0v)
                nc.vector.tensor_copy(W4v[:, :, :, 1:2], w1v)
                # index pair: i0 = floor, i1 = min(floor+1, lim)
                I4 = wk.tile([R, 4 * NB], F32, tag=name + "I4")
                I4v = I4[:, :].rearrange("p (b s c) -> p b s c", s=2, c=2)
                flv = flo.rearrange("p (b s) -> p b s", s=2).unsqueeze(3)
                nc.vector.tensor_copy(I4v[:, :, :, 0:1], flv)
                nc.vector.tensor_scalar(I4v[:, :, :, 1:2], flv, 1.0, lim, OP.add, OP.min)
                return W4, I4

            WX4, XI4 = axis_side(X2, float(W - 1), "x")
            WY4, YI4 = axis_side(Y2, float(H - 1), "y")

            # y-side indices -> flat row base: b*H*W + y*W
            YIr = wk.tile([R, 4 * NB], F32)
            nc.vector.tensor_scalar(
                YIr[:, :], YI4[:, :], float(W), ybase[:, :], OP.mult, OP.add
            )

            # expand to full terms [R, 784] = (bin, sh, cy, sw, cx)
            Wt = wk.tile([R, T], F32)
            Wtv = Wt[:, :].rearrange("p (b h y s x) -> p b h y s x", h=2, y=2, s=2, x=2)
            IDX = wk.tile([R, T], F32)
            IDXv = IDX[:, :].rearrange(
                "p (b h y s x) -> p b h y s x", h=2, y=2, s=2, x=2
            )
            WY4v = WY4[:, :].rearrange("p (b h y) -> p b h y", h=2, y=2).unsqueeze(4).unsqueeze(5)
            YIrv = YIr[:, :].rearrange("p (b h y) -> p b h y", h=2, y=2).unsqueeze(4).unsqueeze(5)
            for k in range(4):
                s, x = k >> 1, k & 1
                nc.vector.tensor_copy(Wtv[:, :, :, :, s : s + 1, x : x + 1], WY4v)
                nc.vector.tensor_copy(IDXv[:, :, :, :, s : s + 1, x : x + 1], YIrv)
            WX4v = WX4[:, :].rearrange("p (b s x) -> p b s x", s=2, x=2).unsqueeze(2).unsqueeze(3)
            XI4v = XI4[:, :].rearrange("p (b s x) -> p b s x", s=2, x=2).unsqueeze(2).unsqueeze(3)
            for j in range(4):
                h, y = j >> 1, j & 1
                dstW = Wtv[:, :, h : h + 1, y : y + 1, :, :]
                dstI = IDXv[:, :, h : h + 1, y : y + 1, :, :]
                nc.vector.tensor_tensor(dstW, dstW, WX4v, OP.mult)
                nc.vector.tensor_tensor(dstI, dstI, XI4v, OP.add)

            # ---- phase B: transpose weights to [128, (n, c)]; fold into
            # per-roi bf16 block-diag masks; build int16 gather indices in
            # dma_gather's 16-lane-wrapped layout.
            #
            # Per 4-roi group the descriptor slot i = col*128 + 16q + l with
            # col = 4c + k (chunk c in 0..5, roi-in-group k) covers bin 8c+q
            # term l of roi 4g+k; col 24 packs bin48 of the 4 rois at
            # partitions 16k+l (trailing 64 slots pad with idx 0).  Idx entry
            # for desc i lives at IDXG[i%16, g*200 + i//16].
            WT = wk.tile([128, R * NCK], F32)
            WTv = WT[:, :].rearrange("p (n c) -> p n c", c=NCK)
            for c in range(NCK):
                psA = pst.tile([128, R], F32, tag="pstr")
                nc.tensor.transpose(psA[:, :], Wt[:, c * 128 : (c + 1) * 128], iden[:, :])
                nc.vector.tensor_copy(WTv[:, :, c : c + 1], psA[:, :].unsqueeze(2))
            # bin48 weights: wt48[16k+l, n] = Wt[n, 768+l] for k = n%4
            psAw = pst.tile([16, R], F32, tag="pstr")
            nc.tensor.transpose(psAw[:, :], Wt[:, 768:784], iden[:, :])
            wt48 = wk.tile([128, R], BF16)
            nc.vector.memset(wt48[:, :], 0.0)
            aw = wk.tile([16, R], BF16)
            nc.vector.tensor_copy(aw[:, :], psAw[:, :])
            awv = aw[:, :].rearrange("p (g k) -> p g k", k=G)
            for k in range(G):
                dst = wt48[16 * k : 16 * (k + 1), :].rearrange(
                    "p (g k) -> p g k", k=G
                )
                nc.sync.dma_start(dst[:, :, k : k + 1], awv[:, :, k : k + 1])
            # all-roi masks: wma[p, n, c, j] = WT[p, n, c] * (j == 8c + p//16)
            # (full 48-bin mask per chunk; 6 chunk matmuls accumulate into one
            # [48, 256] PSUM region since PE outs must start at partition 0)
            wma = wk.tile([128, R * NCK * 48], BF16)
            wmav = wma[:, :].rearrange("p (n c j) -> p n c j", c=NCK, j=48)
            nc.any.tensor_tensor(
                wmav,
                WTv.unsqueeze(3).broadcast_to([128, R, NCK, 48]),
                cm8[:, :].rearrange("p (c j) -> p c j", j=48)
                .unsqueeze(1)
                .broadcast_to([128, R, NCK, 48]),
                OP.mult,
            )

            IDXG = wk.tile([128, NG * MW], I16)
            IDXGv = IDXG[:, :].rearrange("p (g w q) -> p g w q", w=NCOL, q=8)
            for b in range(48):
                c, q = b // 8, b % 8
                psB = pst.tile([16, R], F32, tag="pstr")
                nc.tensor.transpose(psB[:, :], IDX[:, b * TPB : (b + 1) * TPB], iden[:, :])
                src = psB[:, :].rearrange("p (g k) -> p g k", k=G).unsqueeze(3)
                nc.vector.tensor_copy(
                    IDXGv[0:16, :, G * c : G * (c + 1), q : q + 1]
                    .rearrange("p g k q -> p g k q"),
                    src,
                )
            psB48 = pst.tile([16, R], F32, tag="pstr")
            nc.tensor.transpose(psB48[:, :], IDX[:, 768:784], iden[:, :])
            nc.vector.tensor_copy(
                IDXGv[0:16, :, NCOL - 1 : NCOL, 0:G],
                psB48[:, :].rearrange("p (g k) -> p g k", k=G).unsqueeze(2),
            )
            nc.vector.memset(IDXGv[0:16, :, NCOL - 1 : NCOL, G:8], 0)
            # Q7 tx/rx cpus each read their own 16-partition window of the
            # index tensor -> replicate lane group 0 across all 8 groups.
            for grp in range(1, 8):
                nc.sync.dma_start(
                    IDXG[16 * grp : 16 * (grp + 1), :], IDXG[0:16, :]
                )

            # ---- phase C: gather + weighted reduce ----
            out_r = out_d.ap().rearrange("r (b c) -> r b c", c=CH)
            # bin48 staging: stage[k, g, :] = bin48 of roi 4g+k (one DMA/pass)
            stage = wk.tile([G, NG * CH], BF16)
            stagev = stage[:, :].rearrange("p (g c) -> p g c", c=CH)
            out48_r = out_d.ap().rearrange(
                "(g k) (b c) -> k g b c", k=G, c=CH
            )[:, :, NB - 1, :]
            from contextlib import nullcontext
            loop_cm = tc.For_i(0, reps, 1) if reps > 1 else nullcontext()
            with loop_cm:
              for g in range(NG * unroll):
                  g = g % NG
                  gt = gp.tile([128, NCOL * CH], BF16)
                  dest = gt[:, :].rearrange("p (j f) -> p j f", f=CH)
                  for sub, (c0, c1) in enumerate([(0, 8), (8, 16), (16, 24), (24, 25)]):
                      nsub = (c1 - c0) * 128
                      nc.gpsimd.dma_gather(
                          dest[:, c0:c1, :],
                          data.ap(),
                          IDXG[:, g * MW + c0 * 8 : g * MW + c1 * 8],
                          nsub,
                          nsub,
                          CH,
                          queue_num=sub,
                      )
                  gv = gt[:, :].rearrange("p (j f) -> p j f", f=CH)
                  # bin48 of the 4 rois via one shared matmul
                  ps48 = pss.tile([G, CH], F32)
                  nc.tensor.matmul(
                      ps48[:, :], wt48[:, g * G : (g + 1) * G],
                      gv[:, NCOL - 1, :], start=True, stop=True,
                  )
                  nc.vector.tensor_copy(stagev[:, g : g + 1, :], ps48[:, :].unsqueeze(1))
                  ob4 = obp.tile([48, G * CH], BF16)
                  ob4v = ob4[:, :].rearrange("p (k c) -> p k c", c=CH)
                  for k in range(G):
                      n = g * G + k
                      ps = psp.tile([48, CH], F32)
                      for c in range(NCK):
                          nc.tensor.matmul(
                              ps[:, :],
                              wmav[:, n, c, :],
                              gv[:, G * c + k, :],
                              start=(c == 0), stop=(c == NCK - 1),
                          )
                      if k % 2 == 0:
                          nc.vector.tensor_copy(ob4v[:, k, :], ps[:, :])
                      else:
                          nc.scalar.activation(
                              ob4v[:, k, :], ps[:, :],
                              mybir.ActivationFunctionType.Copy,
                          )
                  nc.sync.dma_start(
                      out_r[g * G : (g + 1) * G, 0:48, :].rearrange(
                          "r b c -> b r c"
                      ),
                      ob4[:, :],
                  )
              nc.sync.dma_start(out48_r, stagev)

    nc.finalize()
    return nc


def host_constants():
    iopw = np.tile((np.arange(NB) % P).astype(np.float32), (R, 1))
    ioph = np.tile((np.arange(NB) // P).astype(np.float32), (R, 1))
    iden = np.eye(R, dtype=np.float32)
    cm8 = np.zeros((128, NCK * 48), dtype=np.float32)
    for c in range(NCK):
        for p in range(128):
            cm8[p, c * 48 + 8 * c + p // 16] = 1.0
    return {"iota_pw": iopw, "iota_ph": ioph, "identity": iden, "cm8": cm8}


_cache = {}


def _program():
    if "nc" not in _cache:
        _cache["nc"] = build_program()
    return _cache["nc"]


def make_in_maps(data, rois, offset):
    data = np.asarray(data, dtype=np.float32)
    rois = np.asarray(rois, dtype=np.float32)
    offset = np.asarray(offset, dtype=np.float32)
    data_t = (
        np.ascontiguousarray(data.transpose(0, 2, 3, 1))
        .reshape(NPX, CH)
        .astype(ml_dtypes.bfloat16)
    )
    consts = host_constants()
    in_maps = []
    for c in range(N_CORES):
        sl = slice(c * R, (c + 1) * R)
        m = {
            "data_t": data_t,
            "rois": rois[sl],
            "offs": offset[sl].reshape(R, 2 * NB),
        }
        m.update(consts)
        in_maps.append(m)
    return in_maps


def run(data, rois, offset, **spmd_kwargs):
    n_rois = np.asarray(rois).shape[0]
    in_maps = make_in_maps(data, rois, offset)
    res = run_bass_kernel_spmd(
        _program(), in_maps, core_ids=list(range(N_CORES)), **spmd_kwargs
    )
    outs = np.concatenate([res.results[c]["out"] for c in range(N_CORES)], axis=0)
    out = (
        outs.astype(np.float32)
        .reshape(n_rois, NB, CH)
        .transpose(0, 2, 1)
        .reshape(n_rois, CH, P, P)
    )
    return np.ascontiguousarray(out), res


def kernel(data, rois, offset):
    out, _ = run(data, rois, offset)
    return out
